# revision 6
# baseline (speedup 1.0000x reference)
"""DigitCaps dynamic-routing kernel for 8 TRN2 NeuronCores.

Algorithm (never materializes u_hat):
  Shard over capsules C=96 -> 12 per core (makes every routing step
  core-local: softmax over R is per-capsule, the batch-mean a_ij needs
  no cross-core reduction -> zero collectives).

  Per core, with K = R*I = 3840 the flattened contraction dim:

  s-phase (k-layout A: k = i*2*96 blocks, chunk t <-> i=t//2, r=(t%2)*96+p):
    s[b,(c,o)]    = sum_k wc[k,(c,o)] * xt[k, b]           (PE, 80 mm)
    wc[k,(c,o)]   = cB[k,(c,o)] * wt[k,(c,o)]              (DVE bf16 2x)
    v = squash(s)                                          (ACT+DVE, small)

  a-phase (k-layout B: k = r*20 + i, (c,o) on partitions):
    M'[(c,o), k]  = sum_b v[b,(c,o)] * x[b, k]             (PE, 32 mm N=512,
                                                            stationary=v reused)
    P = W2 .* M'  (ACT copies psum->bf16, DVE 2x muls)
    Q[(c,o), r]   = sum_i P                                (DVE reduce, step-1)
    a[c, r]       = SEL.T @ Q  (SEL = delta(c)/B)          (PE, tiny)
    b_ij += a; c_ij = softmax_r(b_ij)  (two [6,192] half-tiles)

  Matmul inputs bf16 (measured ~3.8e-3 global rel err vs f32 reference),
  routing/softmax/squash math in f32.
"""

import numpy as np
import ml_dtypes

import concourse.bass as bass
import concourse.mybir as mybir
from concourse import tile
from concourse.vector_clock import ScopedClock

B, R, C, O, I = 256, 192, 96, 16, 20
NCORES = 8
CL = C // NCORES          # 12 capsules per core
M = CL * O                # 192 = (c,o) free dim per core
KC = R // 2               # 96 = k-chunk size for s-phase (partition dim)
NK = 2 * I                # 40 s-phase k-chunks
KT = R * I                # 3840 total contraction
NJ = 8                    # a-phase moving slices (3840 = 8 * 480)
JW = KT // NJ             # 480
NITER = 3

F32 = mybir.dt.float32
BF16 = mybir.dt.bfloat16
AF = mybir.ActivationFunctionType
ALU = mybir.AluOpType
AX = mybir.AxisListType


class _TC(tile.TileContext):
    """TileContext whose exit drain splits its semaphore waits across
    chained SP nops -- the walrus in this container caps sync-waits per
    CTRL instruction at 1."""

    def _drain_and_barrier(self, tick_clock, wait_clock):
        nc = self.nc
        lead = nc.sync.nop(nofuse=True)
        wait_clock.add_sem_waits(
            lead.ins, ScopedClock({None: tick_clock.global_clock})
        )
        si = lead.ins.sync_info
        waits = list(si.on_wait) if (si and si.on_wait) else []
        if len(waits) > 1:
            si.on_wait = waits[:1]
            # distribute the remaining waits round-robin across all engine
            # sequencers -- they run in parallel and the all_engine_barrier
            # below joins them, so this is ~5x faster than a serial SP chain
            engs = [nc.sync, nc.vector, nc.scalar, nc.tensor, nc.gpsimd]
            for k, w in enumerate(waits[1:]):
                n = engs[k % len(engs)].nop(nofuse=True)
                nsi = n.ins.sync_info
                if nsi is None:
                    n.ins.sync_info = mybir.SyncInfo(on_wait=[w], on_update=[])
                else:
                    nsi.on_wait = [w]
        nc.sync.drain()
        nc.all_engine_barrier()
        assert self.sems is not None
        popped = nc._tile_sem_poison_stack.pop()
        assert popped is self._sem_poison
        # final barrier elided: the clears run on gpsimd's stream and
        # end-of-execution engine completion already covers them
        nc.clear_and_free_semaphores(list(self.sems.allocated().values()))


def _split_multi_waits(nc):
    """The walrus build in this container caps sync-waits at 1 per
    instruction. Hoist extra waits onto same-engine nops inserted just
    before the offending instruction (engine sequencers are serial, so
    chained single-wait nops are semantically identical)."""
    cur = nc.cur_bb.bb

    def make_nop(engine):
        bi = nc.engines[engine].nop(nofuse=True)
        lst = cur.instructions
        assert lst[-1].name == bi.ins.name
        cur.instructions = lst[:-1]
        return bi.ins

    for f in nc.m.functions:
        for bb in f.blocks:
            insts = bb.instructions
            out = []
            changed = False
            for ins in insts:
                si = ins.sync_info
                waits = list(si.on_wait) if (si and si.on_wait) else []
                if len(waits) > 1:
                    changed = True
                    for w in waits[:-1]:
                        nop = make_nop(ins.engine)
                        nsi = nop.sync_info
                        if nsi is None:
                            nop.sync_info = mybir.SyncInfo(
                                on_wait=[w], on_update=[]
                            )
                        else:
                            nsi.on_wait = [w]
                        out.append(nop)
                    si.on_wait = waits[-1:]
                out.append(ins)
            if changed:
                bb.instructions = out


def _sel_const():
    # SEL[p, j] = 1/B where p//16 == j: contracts o within a (c,o)-group
    # of 96 partitions down to 6 capsules, folding the batch-mean scale.
    sel = np.zeros((KC, CL // 2), dtype=ml_dtypes.bfloat16)
    for p in range(KC):
        sel[p, p // O] = 1.0 / B
    return sel


def build_nc():
    nc = bass.Bass()
    NW = 4            # wt/xt SBUF tiles (10 chunks each)
    CPW = NK // NW
    # Inputs land as many small pieces, all triggered in strict consumption
    # order on the sync queue: wx (s-phase, needed first) -> xb (M'-phase)
    # -> w2 (P-mult).  Per-piece completion semaphores let the it0 s-matmuls
    # start as soon as the first 5 chunks arrive instead of waiting for the
    # whole 3.4MB of wx.
    wx_d = nc.declare_dram_parameter("wx", [2 * NW, KC, CPW // 2, M + B], BF16, isOutput=False)
    xb_d = nc.declare_dram_parameter("xb", [4, 128, 2, KT // 4], BF16, isOutput=False)
    w2_d = nc.declare_dram_parameter("w2", [2, 2, KC, KT // 2], BF16, isOutput=False)
    out_d = nc.declare_dram_parameter("out", [B, M], F32, isOutput=True)
    ident_d = nc.inline_tensor(np.eye(CL, dtype=np.float32), "ident")
    sel_d = nc.inline_tensor(_sel_const(), "sel")

    with _TC(nc) as tc:
        with (
            tc.tile_pool(name="big", bufs=1) as big,
            tc.tile_pool(name="wcp", bufs=1) as wcp,
            tc.tile_pool(name="sm", bufs=2) as sm,
            tc.tile_pool(name="ps_s", bufs=1, space="PSUM") as ps_s,
            tc.tile_pool(name="ps_m", bufs=2, space="PSUM") as ps_m,
            tc.tile_pool(name="ps_t", bufs=2, space="PSUM") as ps_t,
        ):
            # ---- persistent SBUF tensors -------------------------------
            wx_t = [big.tile([KC, CPW, M + B], BF16, tag=f"wx{j}", name=f"wx{j}")
                    for j in range(NW)]
            xb_t = [big.tile([128, 2, KT // 2], BF16, tag=f"xb{j}", name=f"xb{j}")
                    for j in range(2)]
            w2_t = [big.tile([KC, KT], BF16, tag=f"w2{g}", name=f"w2{g}")
                    for g in range(2)]
            ident = big.tile([CL, CL], F32, tag="ident")
            sel = big.tile([KC, CL // 2], BF16, tag="sel")
            bT = [big.tile([CL // 2, R], F32, tag=f"bT{h}", name=f"bT{h}")
                  for h in range(2)]

            nc.scalar.dma_start(ident[:], ident_d[:])
            nc.scalar.dma_start(sel[:], sel_d[:])
            HC = CPW // 2
            for j in range(NW):
                for h in range(2):
                    nc.sync.dma_start(
                        wx_t[j][:, h * HC:(h + 1) * HC, :], wx_d[2 * j + h]
                    )
            for q in range(2):
                for h in range(2):
                    nc.sync.dma_start(
                        xb_t[q][:, :, h * 960:(h + 1) * 960], xb_d[2 * q + h]
                    )
            for g in range(2):
                for h in range(2):
                    nc.sync.dma_start(
                        w2_t[g][:, h * (KT // 2):(h + 1) * (KT // 2)], w2_d[g][h]
                    )

            def wt_c(t):   # wt chunk t -> [96, 192] AP
                return wx_t[t // CPW][:, t % CPW, 0:M]

            def xt_c(t, bt):  # [96, 128] lhsT for s-matmul
                return wx_t[t // CPW][:, t % CPW, M + bt * 128:M + (bt + 1) * 128]

            def xb_s(bt, j):  # [128, 480] moving slice for M'-matmul
                q, r0 = divmod(j * JW, KT // 2)
                return xb_t[q][:, bt, r0:r0 + JW]

            def w2_s(g, j):  # [96, 480] W2 slice
                return w2_t[g][:, j * JW:(j + 1) * JW]

            # wc lives in one chunked pool (bf16)
            wc_t = [wcp.tile([KC, CPW, M], BF16, tag=f"wc{j}", name=f"wc{j}")
                    for j in range(NW)]

            def wc_c(t):
                return wc_t[t // CPW][:, t % CPW, :]

            co = dict(o=O)

            def warm(anchor):
                # 1x1 matmul anchored on `anchor` ([1, 1] SBUF AP) -- keeps
                # the PE HAM window busy through DVE/ACT stretches so matmul
                # phases restart at 2.4 GHz.
                wp = ps_t.tile([1, 1], F32, tag="pst", name="warmp")
                nc.tensor.matmul(wp[:], anchor, anchor, start=True, stop=True)


            vT = None
            for it in range(NITER):
                last = it == NITER - 1

                # ---- s-matmul: s[b,(c,o)] accumulated over 40 chunks ---
                # it0: N=192 full-width MMs from wt; later iterations run
                # per capsule-half so half-0's matmuls overlap half-1's
                # routing math on DVE.
                s_ps = [ps_s.tile([128, M], F32, tag=f"s{bt}", name=f"s{bt}")[:]
                        for bt in range(2)]
                if it == 0:
                    # chunk-outer so the matmul consumption order matches the
                    # piecewise DMA arrival order (phase is DMA-paced)
                    for t in range(NK):
                        for bt in range(2):
                            nc.tensor.matmul(
                                s_ps[bt],
                                xt_c(t, bt),
                                wt_c(t),
                                start=(t == 0),
                                stop=(t == NK - 1),
                            )
                else:
                    for g in range(2):
                        for bt in range(2):
                            for t in range(NK):
                                nc.tensor.matmul(
                                    s_ps[bt][:, g * KC:(g + 1) * KC],
                                    xt_c(t, bt),
                                    wc_c(t)[:, g * KC:(g + 1) * KC],
                                    start=(t == 0),
                                    stop=(t == NK - 1),
                                )

                # ---- squash (split per capsule-half: M'-half-g and the
                # final output DMA only need their own half of v) ----------
                scale = 1.0 / R if it == 0 else 1.0
                CH = CL // 2
                vT = sm.tile([128, 2, M], BF16, tag="vT")
                vOut = (
                    sm.tile([128, 2, M], F32, tag="vOut", name="vOut")
                    if last
                    else None
                )
                for g2 in range(2):
                    for bt in range(2):
                        s_h = s_ps[bt][:, g2 * KC:(g2 + 1) * KC]
                        sq = sm.tile([128, KC], F32, tag="sq", bufs=4)
                        nc.scalar.activation(sq[:], s_h, AF.Square, scale=scale)
                        n2 = sm.tile([128, CH], F32, tag="n2", bufs=4)
                        nc.vector.reduce_sum(
                            n2[:], sq[:].rearrange("p (c o) -> p c o", **co),
                            axis=AX.X,
                        )
                        # sqrt via exp(0.5*ln(.)) -- keeps every activation in
                        # one ACT table set (no ~2.7us table swaps)
                        lnn = sm.tile([128, CH], F32, tag="lnn", bufs=4)
                        nc.scalar.activation(lnn[:], n2[:], AF.Ln)
                        nrm = sm.tile([128, CH], F32, tag="nrm", bufs=4)
                        nc.scalar.activation(nrm[:], lnn[:], AF.Exp, scale=0.5)
                        den = sm.tile([128, CH], F32, tag="den", bufs=4)
                        nc.scalar.activation(den[:], n2[:], AF.Identity, bias=1.0)
                        rden = sm.tile([128, CH], F32, tag="rden", bufs=4)
                        nc.vector.reciprocal(rden[:], den[:])
                        g = sm.tile([128, CH], F32, tag="g", bufs=4)
                        nc.vector.scalar_tensor_tensor(
                            g[:], nrm[:], scale, rden[:],
                            op0=ALU.mult, op1=ALU.mult,
                        )
                        vdst = (vOut if last else vT)[:, bt, g2 * KC:(g2 + 1) * KC]
                        nc.vector.tensor_tensor(
                            vdst.rearrange("p (c o) -> p c o", **co),
                            s_h.rearrange("p (c o) -> p c o", **co),
                            g[:].to_broadcast([128, CH, O]),
                            op=ALU.mult,
                        )
                        if last:
                            nc.sync.dma_start(
                                out_d[bt * 128:(bt + 1) * 128,
                                      g2 * KC:(g2 + 1) * KC],
                                vOut[:, bt, g2 * KC:(g2 + 1) * KC],
                            )
                if last:
                    break

                # ---- a-phase: M' = v^T x, P = W2.*M', reduce ----------
                pb = sm.tile([KC, 2, KT], BF16, tag="pb")   # P bf16
                for g in range(2):
                    for jp in range(NJ // 2):
                        mps = ps_m.tile([KC, 2, 512], F32, tag="mps")
                        for q in range(2):        # two psum banks per tile
                            for bt in range(2):
                                nc.tensor.matmul(
                                    mps[:, q, 0:JW],
                                    vT[:, bt, g * KC:(g + 1) * KC],
                                    xb_s(bt, 2 * jp + q),
                                    start=(bt == 0),
                                    stop=(bt == 1),
                                )
                        # multiply straight out of PSUM (f32) -- saves the
                        # ACT psum->bf16 copy that used to gate this chain
                        nc.vector.tensor_tensor(
                            pb[:, g, 2 * jp * JW:(2 * jp + 2) * JW].rearrange(
                                'p (q j) -> p q j', q=2
                            ),
                            w2_t[g][:, 2 * jp * JW:(2 * jp + 2) * JW].rearrange(
                                'p (q j) -> p q j', q=2
                            ),
                            mps[:, :, 0:JW],
                            op=ALU.mult,
                        )
                # Per capsule-half: i-reduce -> SEL matmul -> b-update ->
                # softmax -> cB build -> wc-half muls. Emitting both halves'
                # chains before the s-matmuls lets half-0's s-MMs (PE) overlap
                # half-1's routing math (DVE).
                qT = sm.tile([KC, 2, R], BF16, tag="qT")
                cB = sm.tile([KC, 2, M], BF16, tag="cB")
                RQ = R // 4
                for g in range(2):
                    for q in range(4):
                        with nc.allow_low_precision("a_ij steers routing only"):
                            nc.vector.reduce_sum(
                                qT[:, g, q * RQ:(q + 1) * RQ],
                                pb[:, g, q * RQ * I:(q + 1) * RQ * I].rearrange(
                                    "p (r i) -> p r i", i=I
                                ),
                                axis=AX.X,
                            )
                    aps = ps_t.tile([CL // 2, R], F32, tag="pst", name="aps")
                    nc.tensor.matmul(
                        aps[:], sel[:], qT[:, g, :], start=True, stop=True
                    )
                    if it == 0:
                        nc.scalar.copy(bT[g][:], aps[:])
                    else:
                        nc.vector.tensor_tensor(
                            bT[g][:], bT[g][:], aps[:], op=ALU.add
                        )
                    # softmax over r for this half; no max-subtraction --
                    # b_ij is a 2-step sum of batch-mean agreements, bounded
                    # well inside exp's f32 range
                    eT = sm.tile([CL // 2, R], F32, tag="eT")
                    ssum = sm.tile([CL // 2, 1], F32, tag="ssum")
                    nc.scalar.activation(
                        eT[:], bT[g][:], AF.Exp, accum_out=ssum[:]
                    )
                    rs = sm.tile([CL // 2, 1], F32, tag="rs")
                    nc.vector.reciprocal(rs[:], ssum[:])
                    cT = sm.tile([CL // 2, R], F32, tag="cT")
                    nc.scalar.activation(cT[:], eT[:], AF.Copy, scale=rs[:])
                    for par in range(2):
                        cps = ps_t.tile([KC, CL // 2], F32, tag="pst", name="cps")
                        nc.tensor.transpose(
                            cps[:],
                            cT[:, par * KC:(par + 1) * KC],
                            ident[0:CL // 2, 0:CL // 2],
                        )
                        nc.scalar.copy(
                            cB[:, par, g * KC:(g + 1) * KC].rearrange(
                                "p (c o) -> p c o", **co
                            ),
                            cps[:].to_broadcast([KC, CL // 2, O]),
                        )
                    # wc-half = cB-half .* wt-half  (bf16 step-1 -> 2x)
                    for j in range(NW):
                        nc.vector.tensor_tensor(
                            wc_t[j][:, :, g * KC:(g + 1) * KC].rearrange(
                                "p (u par) m -> p u par m", par=2
                            ),
                            wx_t[j][:, :, g * KC:(g + 1) * KC].rearrange(
                                "p (u par) m -> p u par m", par=2
                            ),
                            cB[:, :, g * KC:(g + 1) * KC].unsqueeze(1)
                            .broadcast_to([KC, CPW // 2, 2, KC]),
                            op=ALU.mult,
                        )
    _split_multi_waits(nc)
    return nc


def prep_inputs(x, W, core):
    """Host-side shard prep for one core -> dict of bf16 arrays."""
    bf = ml_dtypes.bfloat16
    cs = core * CL
    # xt[p, t, b]: t = i*2 + par, r = par*96 + p   (k-layout A)
    xt = (
        np.transpose(x, (2, 1, 0))
        .reshape(I, 2, KC, B)
        .transpose(2, 0, 1, 3)
        .reshape(KC, NK, B)
    )
    # wt[p, t, c*16+o]  (k-layout A)
    Ws = W[:, cs:cs + CL]
    wt = (
        Ws.transpose(3, 0, 1, 2)
        .reshape(I, 2, KC, CL, O)
        .transpose(2, 0, 1, 3, 4)
        .reshape(KC, NK, M)
    )
    # xb[pb, bt, k]: k = r*20 + i  (k-layout B, natural x order)
    xb = x.reshape(2, 128, KT).transpose(1, 0, 2)
    # w2[(c,o) % 96, g, k]: rows (c,o), k-layout B
    w2 = (
        Ws.transpose(1, 2, 0, 3)
        .reshape(2, KC, KT)
        .transpose(1, 0, 2)
    )
    NP, CPP = 8, NK // 8        # 8 wx DMA pieces of 5 chunks
    xt = xt.reshape(KC, NP, CPP, B).transpose(1, 0, 2, 3)
    wt = wt.reshape(KC, NP, CPP, M).transpose(1, 0, 2, 3)
    wx = np.concatenate([wt, xt], axis=-1)        # [8, 96, 5, 448]
    xb = xb.reshape(128, 2, 4, KT // 4).transpose(2, 0, 1, 3)  # [4,128,2,960]
    w2 = (
        w2.transpose(1, 0, 2)                      # [2, 96, 3840]
        .reshape(2, KC, 2, KT // 2)
        .transpose(0, 2, 1, 3)                     # [2, 2, 96, 1920]
    )
    return {
        "wx": np.ascontiguousarray(wx).astype(bf),
        "xb": np.ascontiguousarray(xb).astype(bf),
        "w2": np.ascontiguousarray(w2).astype(bf),
    }


_CACHED_NC = None


def kernel(x, W):
    from concourse.bass_utils import run_bass_kernel_spmd

    global _CACHED_NC
    x = np.asarray(x, dtype=np.float32)
    W = np.asarray(W, dtype=np.float32)
    if _CACHED_NC is None:
        _CACHED_NC = build_nc()
    nc = _CACHED_NC
    in_maps = [prep_inputs(x, W, core) for core in range(NCORES)]
    res = run_bass_kernel_spmd(nc, in_maps, list(range(NCORES)))
    v = np.empty((B, C, O), dtype=np.float32)
    for core in range(NCORES):
        v[:, core * CL:(core + 1) * CL, :] = (
            res.results[core]["out"].reshape(B, CL, O)
        )
    return v

